# revision 2
# baseline (speedup 1.0000x reference)
import numpy as np
import jax
import jax.numpy as jnp
from functools import partial

# nn_CrossWindowBlock — hardcoded dims (from spec)
DIM = 384; H = 64; W = 64; WS = 4; NH = 6; SHIFT = 2
L = WS * WS
DH = DIM // NH
SCALE = DH ** -0.5
MLP_H = int(DIM * 4.0)
NW = (H // WS) * (W // WS)
NCORES = 8


def _build_mask():
    img = np.zeros((H, W))
    slices = [slice(0, -WS), slice(-WS, -SHIFT), slice(-SHIFT, None)]
    cnt = 0
    for hs in slices:
        for ws_ in slices:
            img[hs, ws_] = cnt
            cnt += 1
    mw = img.reshape(H // WS, WS, W // WS, WS).transpose(0, 2, 1, 3).reshape(-1, L)
    am = mw[:, None, :] - mw[:, :, None]
    am = np.where(am != 0, -100.0, 0.0).astype(np.float32)
    return am  # (nW, L, L)


_ATTN_MASK_NP = _build_mask()


def _ln(x, g, b):
    m = x.mean(-1, keepdims=True)
    v = ((x - m) ** 2).mean(-1, keepdims=True)
    return (x - m) / jnp.sqrt(v + 1e-5) * g + b


def _win(x, B):
    x = x.reshape(B, H // WS, WS, W // WS, WS, DIM)
    x = x.transpose(0, 1, 3, 2, 4, 5)
    return x.reshape(B * NW, L, DIM)


def _unwin(xw, B):
    x = xw.reshape(B, H // WS, W // WS, WS, WS, DIM)
    x = x.transpose(0, 1, 3, 2, 4, 5)
    return x.reshape(B, H * W, DIM)


def _ln2(x2, g, b):
    # x2: (M, C) — keep every reduce 2-D over the last axis
    m = jnp.mean(x2, axis=1, keepdims=True)
    xc = x2 - m
    v = jnp.mean(xc * xc, axis=1, keepdims=True)
    return xc * jax.lax.rsqrt(v + 1e-5) * g + b


def _block(x_main, x_mpmt, g_q, b_q, g_km, b_km, g_kp, b_kp,
           Wq, bq, Wk, bk, Wv, bv, Wo, bo, g_mlp, b_mlp, W1, b1, W2, b2,
           mask):
    B, N, C = x_main.shape
    x2_main = x_main.reshape(B * N, C)
    x2_mpmt = x_mpmt.reshape(B * N, C)
    q = _ln2(x2_main, g_q, b_q).reshape(B, N, C)
    km = _ln2(x2_main, g_km, b_km).reshape(B, N, C)
    kp = _ln2(x2_mpmt, g_kp, b_kp).reshape(B, N, C)

    roll = lambda t: jnp.roll(t.reshape(B, H, W, C), (-SHIFT, -SHIFT),
                              axis=(1, 2)).reshape(B, N, C)
    q, km, kp = roll(q), roll(km), roll(kp)

    q_w = _win(q, B)
    kv_w = jnp.concatenate([_win(km, B), _win(kp, B)], axis=1)

    def heads(x, Wm, bm):
        y = x.reshape(-1, C) @ Wm + bm
        y = y.reshape(-1, x.shape[1], NH, DH)
        return y.transpose(0, 2, 1, 3)

    Q = heads(q_w, Wq, bq)    # (B*nW, h, L, dh)
    K = heads(kv_w, Wk, bk)   # (B*nW, h, 2L, dh)
    V = heads(kv_w, Wv, bv)

    scores = jnp.matmul(Q, jnp.swapaxes(K, -1, -2)) * SCALE  # (B*nW, h, L, 2L)
    mask2 = jnp.concatenate([mask, mask], axis=-1)           # (nW, L, 2L)
    scores = scores.reshape(B, NW, NH, L, 2 * L) + mask2[None, :, None]

    # manual softmax, 2-D reduce; logits are small and mask is -100 so no
    # max-subtraction is needed for fp32 stability
    s2 = scores.reshape(B * NW * NH * L, 2 * L)
    e = jnp.exp(s2)
    attn = e * (1.0 / jnp.sum(e, axis=1, keepdims=True))
    attn = attn.reshape(B * NW, NH, L, 2 * L)

    out = jnp.matmul(attn, V)                                # (B*nW, h, L, dh)
    out = out.transpose(0, 2, 1, 3).reshape(B * NW, L, C)
    y = _unwin(out, B)
    y = jnp.roll(y.reshape(B, H, W, C), (SHIFT, SHIFT), axis=(1, 2)).reshape(B, N, C)

    x = x_main.reshape(B * N, C) + (y.reshape(B * N, C) @ Wo + bo)
    xm = _ln2(x, g_mlp, b_mlp)
    h1 = jax.nn.gelu(xm @ W1 + b1, approximate=False)
    x = x + (h1 @ W2 + b2)
    return x.reshape(B, N, C)


_PMAPPED = None


def _get_pmapped():
    global _PMAPPED
    if _PMAPPED is None:
        devs = jax.devices()[:NCORES]
        # batch-shard args 0,1; replicate the 20 params + mask
        in_axes = (0, 0) + (None,) * 21
        _PMAPPED = jax.pmap(_block, in_axes=in_axes, devices=devs)
    return _PMAPPED


_PARAM_ORDER = ['g_q', 'b_q', 'g_km', 'b_km', 'g_kp', 'b_kp',
                'Wq', 'bq', 'Wk', 'bk', 'Wv', 'bv', 'Wo', 'bo',
                'g_mlp', 'b_mlp', 'W1', 'b1', 'W2', 'b2']


def kernel(**inputs):
    x_main = np.asarray(inputs['x_main'], dtype=np.float32)
    x_mpmt = np.asarray(inputs['x_mpmt'], dtype=np.float32)
    B = x_main.shape[0]
    per = B // NCORES  # 2 images per core

    xm_sh = x_main.reshape(NCORES, per, H * W, DIM)
    xp_sh = x_mpmt.reshape(NCORES, per, H * W, DIM)

    params = [np.asarray(inputs[k], dtype=np.float32) for k in _PARAM_ORDER]
    fn = _get_pmapped()
    out = fn(xm_sh, xp_sh, *params, _ATTN_MASK_NP)
    out = np.asarray(out).reshape(B, H * W, DIM).astype(np.float32)
    return out


if __name__ == '__main__':
    # smoke test with random data
    rng = np.random.default_rng(0)
    ins = {
        'x_main': rng.standard_normal((16, H * W, DIM), dtype=np.float32),
        'x_mpmt': rng.standard_normal((16, H * W, DIM), dtype=np.float32),
        'g_q': np.ones(DIM, np.float32), 'b_q': np.zeros(DIM, np.float32),
        'g_km': np.ones(DIM, np.float32), 'b_km': np.zeros(DIM, np.float32),
        'g_kp': np.ones(DIM, np.float32), 'b_kp': np.zeros(DIM, np.float32),
        'Wq': rng.standard_normal((DIM, DIM), dtype=np.float32) * 0.02,
        'bq': np.zeros(DIM, np.float32),
        'Wk': rng.standard_normal((DIM, DIM), dtype=np.float32) * 0.02,
        'bk': np.zeros(DIM, np.float32),
        'Wv': rng.standard_normal((DIM, DIM), dtype=np.float32) * 0.02,
        'bv': np.zeros(DIM, np.float32),
        'Wo': rng.standard_normal((DIM, DIM), dtype=np.float32) * 0.02,
        'bo': np.zeros(DIM, np.float32),
        'g_mlp': np.ones(DIM, np.float32), 'b_mlp': np.zeros(DIM, np.float32),
        'W1': rng.standard_normal((DIM, MLP_H), dtype=np.float32) * 0.02,
        'b1': np.zeros(MLP_H, np.float32),
        'W2': rng.standard_normal((MLP_H, DIM), dtype=np.float32) * 0.02,
        'b2': np.zeros(DIM, np.float32),
    }
    out = kernel(**ins)
    print('out', out.shape, out.dtype, float(np.abs(out).max()))
